# revision 28
# baseline (speedup 1.0000x reference)
"""Distributed Trainium2 kernel for ArticulatoryMetricLoss.

loss = mean_{i != j} ((||e_i||^2 + ||e_j||^2 - 2 e_i.e_j) - art_dist[i, j])^2

Strategy (8 NeuronCores), exploiting d2's symmetry (d2_ij == d2_ji):
  - The 8x8 grid of 512x512 (i, j) blocks is covered by its 36 unordered
    block-units (8 diagonal + 28 pairs). Each off-diagonal unit's d2 block is
    computed ONCE and consumed against BOTH art orientations (a_ij and a_ji),
    halving the matmul work. Units are split into 144 [128-j x 512-i]
    sub-jobs, 18 per core (diag unit + gap-1..3 pairs + half of a gap-4
    pair) - every core runs the IDENTICAL graph; which global blocks a core
    works on is decided purely by how the host packs its input buffers.
  - Per sub-job: psum[j, i] = sum_k E^T[k, j] * (-2 E_blk^T)[k, i] over 6
    bf16 k-tiles + a K=1 augmented matmul adding s_i (locally computed
    norms - no collective anywhere).
  - Per pass (1 for diag, 2 for pairs): DVE scalar_tensor_tensor computes
    u = psum - art with a fused free-dim sum (A2), ACT Square computes the
    fused sum of squares (A1).
  - The s_j side of d2 enters only through the algebraic expansion
      sum_i (u + s_j)^2 = A1 + 2 s_j A2 + 512 s_j^2,
    evaluated ON THE HOST from the device outputs (A1, A2, and the
    device-computed bf16 norms) during unsharding. No device collective,
    no tail dependency.

Numerics: bf16 quantization of E, fp8(e4m3) quantization of art, and the
symmetric decomposition give ~1.5e-5 relative error on the final scalar
(validated against the fp32 reference in numpy). Diagonal (i == j) terms
are ~0 by construction (consistent quantized norms and gram) and are simply
included; their contribution is ~1e-10 relative.
"""

import os
import sys
from contextlib import ExitStack

import numpy as np

for _p in ("/opt/trn_rl_repo", "/root/.axon_site/_ro/trn_rl_repo"):
    if os.path.isdir(_p) and _p not in sys.path:
        sys.path.insert(0, _p)

import ml_dtypes

import concourse.tile as tile
from concourse import bacc, mybir
from concourse.bass_utils import run_bass_kernel_spmd

B = 4096          # rows/cols of the pairwise matrix
D = 768           # embedding dim
NCORES = 8
BLK = 512         # i/j block size (8x8 block grid)
P = 128           # SBUF partitions
KT = D // P       # 6 contraction tiles
NSUB = 18         # sub-jobs per core
NPASS = 32        # DVE/ACT passes per core
PAIRS = B * (B - 1)

BF16 = mybir.dt.bfloat16
F32 = mybir.dt.float32
F8 = mybir.dt.float8e4

# compile-time per-core structure: (lhs_slot, n_passes) for each sub-job,
# in emission order: t0..11 gap-1..3 pairs (2 passes), t12..13 half of the
# gap-4 pair (aux lhs block, 2 passes), t14..17 diag unit (1 pass, last so
# the pipeline tail is cheap).
SUB_STRUCT = [(0, 2)] * 12 + [(1, 2)] * 2 + [(0, 1)] * 4

_CACHED = {}


def subjobs(c):
    """Host-side per-core sub-job table: (bi, bj, jt, npass).

    Must stay in sync with SUB_STRUCT: bi == c for t0..15 (lhs slot 0),
    bi == c % 4 for t16..17 (lhs slot 1)."""
    jobs = []
    for d in (1, 2, 3):
        for jt in range(4):
            jobs.append((c, (c + d) % 8, jt, 2))
    p = c % 4
    for q in range(2):
        jt = q if c < 4 else q + 2
        jobs.append((p, p + 4, jt, 2))
    for jt in range(4):
        jobs.append((c, c, jt, 1))
    return jobs


def build_graph():
    nc = bacc.Bacc("TRN2", target_bir_lowering=False, debug=False, num_devices=NCORES)

    # per-partition-contiguous packed layouts (fat DMA descriptors)
    lhs_d = nc.dram_tensor("lhs", [P, 2 * KT * BLK], BF16, kind="ExternalInput")
    rhs_d = nc.dram_tensor("rhs", [P, NSUB * KT * P], BF16, kind="ExternalInput")
    esl_d = nc.dram_tensor("eslab", [P, 2 * 4 * D], BF16, kind="ExternalInput")
    art_d = nc.dram_tensor("art", [P, NPASS * BLK], F8, kind="ExternalInput")
    idn_d = nc.dram_tensor("ident", [P, P], F32, kind="ExternalInput")
    a1_d = nc.dram_tensor("a1", [P, NSUB], F32, kind="ExternalOutput")
    a2_d = nc.dram_tensor("a2", [P, NPASS], F32, kind="ExternalOutput")
    sn_d = nc.dram_tensor("snorm", [8, P], BF16, kind="ExternalOutput")

    ART_CH = 4096   # art chunk width (4KB fp8 rows), 8 passes per chunk
    N_ART = NPASS * BLK // ART_CH  # 4
    RHS_CH = 6 * KT * P  # six sub-jobs of stationary columns (9KB rows)

    with tile.TileContext(nc) as tc, ExitStack() as ctx:
        const_pool = ctx.enter_context(tc.tile_pool(name="const", bufs=1))
        rhs_pool = ctx.enter_context(tc.tile_pool(name="rhs", bufs=1))
        lhs_pool = ctx.enter_context(tc.tile_pool(name="lhs", bufs=1))
        art_pool = ctx.enter_context(tc.tile_pool(name="art", bufs=1))
        u_pool = ctx.enter_context(tc.tile_pool(name="u", bufs=8))
        scr_pool = ctx.enter_context(tc.tile_pool(name="scr", bufs=2))
        acc_pool = ctx.enter_context(tc.tile_pool(name="acc", bufs=1))
        psum_pool = ctx.enter_context(tc.tile_pool(name="psum", bufs=7, space="PSUM"))
        psx_pool = ctx.enter_context(tc.tile_pool(name="psx", bufs=1, space="PSUM"))

        # ---- bulk loads, issue order ~= consumption order.
        # sync (HWDGE): first rhs sub-job chunks, lhs/esl slot 0, ident, rest
        # of rhs. gpsimd (SWDGE): esl slot 1, art (fp8), lhs slot 1.
        rhs_t = [None] * 3

        def load_rhs(g, eng):
            rt = rhs_pool.tile([P, RHS_CH], BF16, tag=f"rhs{g}", name=f"rhs{g}")
            eng.dma_start(rt[:], rhs_d[:, g * RHS_CH : (g + 1) * RHS_CH])
            rhs_t[g] = rt

        # few fat transfers: each dma_start fans over all 16 SDMA engines and
        # transfers are serial per trigger stream (fixed ~0.6us HWDGE / ~2us
        # SWDGE each), so consolidation beats chunking.
        lhs_t = []                              # 2 slots, one transfer each
        for L in range(2):
            lt = lhs_pool.tile([P, KT * BLK], BF16, tag=f"lhsL{L}", name=f"lhsL{L}")
            lhs_t.append(lt)
        esl_t = []
        for L in range(2):
            et = lhs_pool.tile([P, 4 * D], BF16, tag=f"eslL{L}", name=f"eslL{L}")
            esl_t.append(et)
        nc.sync.dma_start(esl_t[0][:], esl_d[:, 0 : 4 * D])
        # split the first rhs group and lhs slot so batch-0 matmuls start
        # as soon as the first halves land
        rt0 = rhs_pool.tile([P, RHS_CH], BF16, tag="rhs0", name="rhs0")
        rhs_t[0] = rt0
        nc.sync.dma_start(rt0[:, : RHS_CH // 2], rhs_d[:, 0 : RHS_CH // 2])
        nc.sync.dma_start(
            lhs_t[0][:, : KT * BLK // 2], lhs_d[:, 0 : KT * BLK // 2]
        )
        nc.sync.dma_start(
            rt0[:, RHS_CH // 2 :], rhs_d[:, RHS_CH // 2 : RHS_CH]
        )
        nc.sync.dma_start(
            lhs_t[0][:, KT * BLK // 2 :], lhs_d[:, KT * BLK // 2 : KT * BLK]
        )
        load_rhs(1, nc.sync)                    # sub-jobs 6-11
        load_rhs(2, nc.sync)                    # sub-jobs 12-17
        nc.sync.dma_start(lhs_t[1][:], lhs_d[:, KT * BLK : 2 * KT * BLK])

        ident = const_pool.tile([P, P], F32)
        nc.scalar.dma_start(ident[:], idn_d[:])

        nc.gpsimd.dma_start(esl_t[1][:], esl_d[:, 4 * D : 8 * D])
        art_t = []
        for ch in range(N_ART):
            at = art_pool.tile([P, ART_CH], F8, tag=f"art{ch}", name=f"art{ch}")
            nc.gpsimd.dma_start(at[:], art_d[:, ch * ART_CH : (ch + 1) * ART_CH])
            art_t.append(at)

        def rhs_view(t, k):  # stationary [128, 128] for sub-job t, k-tile k
            off = (t % 6) * KT * P + k * P
            return rhs_t[t // 6][:, off : off + P]

        def lhs_view(L, k):  # moving [128, 512] for lhs slot L, k-tile k
            return lhs_t[L][:, k * BLK : (k + 1) * BLK]

        def esl_view(L, m):  # [128, 768] row-block m of lhs slot L's block
            return esl_t[L][:, m * D : (m + 1) * D]

        def art_view(pi):  # [128, 512] art tile for pass pi
            ch = (pi * BLK) // ART_CH
            off = (pi * BLK) % ART_CH
            return art_t[ch][:, off : off + BLK]

        # ---- norms: ACT square-accum per row-block, transpose via identity
        # matmul, bounce through DRAM to get bf16 [1, 512] aug rows. All local.
        s_sq = acc_pool.tile([P, 8], F32)
        sT_bfs = []
        s_rows = []
        warm = scr_pool.tile([P, 4], BF16, name="warm")
        nc.scalar.activation(   # prewarm the ACT Square table early
            warm[0:1, 0:1], warm[0:1, 1:2], mybir.ActivationFunctionType.Square
        )
        for L in range(2):
            for m in range(4):
                so = scr_pool.tile([P, D], BF16, tag="scr", name=f"sq{L}{m}")
                nc.vector.scalar_tensor_tensor(
                    out=so[:],
                    in0=esl_view(L, m),
                    scalar=0.0,
                    in1=esl_view(L, m),
                    op0=mybir.AluOpType.add,
                    op1=mybir.AluOpType.mult,
                    accum_out=s_sq[:, L * 4 + m : L * 4 + m + 1],
                )
            psum4 = psx_pool.tile([4, P], F32, tag="px", name=f"psum4{L}")
            nc.tensor.matmul(
                psum4[:],
                s_sq[:, L * 4 : (L + 1) * 4],
                ident[:],
                start=True,
                stop=True,
            )
            sT_bf = const_pool.tile([4, P], BF16, name=f"sTbf{L}")
            sT_bfs.append(sT_bf)
            nc.vector.tensor_copy(sT_bf[:], psum4[:])
            sr = const_pool.tile([1, BLK], BF16, name=f"srow{L}")
            nc.scalar.dma_start(sr[:], sT_bf[:])  # SBUF->SBUF flatten
            s_rows.append(sr)

        ones_lhs = const_pool.tile([1, P], BF16)
        nc.vector.memset(ones_lhs[:], 1.0)

        # ---- main loop over 18 sub-jobs in PSUM-sized batches
        A1 = acc_pool.tile([P, NSUB], F32)
        A2 = acc_pool.tile([P, NPASS], F32)
        pass_of = []  # pass index base per sub-job
        pi = 0
        for t in range(NSUB):
            pass_of.append(pi)
            pi += SUB_STRUCT[t][1]
        assert pi == NPASS

        NBATCH = 7
        for b0 in range(0, NSUB, NBATCH):
            batch = range(b0, min(b0 + NBATCH, NSUB))
            psums = {}
            for k in range(KT):
                for t in batch:
                    if k == 0:
                        psums[t] = psum_pool.tile(
                            [P, BLK], F32, tag="ps", name=f"ps{t}"
                        )
                    nc.tensor.matmul(
                        psums[t][:],
                        rhs_view(t, k),
                        lhs_view(SUB_STRUCT[t][0], k),
                        start=(k == 0),
                        stop=False,
                    )
            for t in batch:
                # += 1 * s_i along the free dim (local norms)
                nc.tensor.matmul(
                    psums[t][:],
                    ones_lhs[:],
                    s_rows[SUB_STRUCT[t][0]][:],
                    start=False,
                    stop=True,
                )
            for t in batch:
                npass = SUB_STRUCT[t][1]
                u = u_pool.tile([P, npass * BLK], F32, tag="u", name=f"u{t}")
                for q in range(npass):
                    p_i = pass_of[t] + q
                    # u_q = psum - art_q ; A2[:, pi] = sum_i(u_q)
                    nc.vector.scalar_tensor_tensor(
                        out=u[:, q * BLK : (q + 1) * BLK],
                        in0=psums[t][:],
                        scalar=0.0,
                        in1=art_view(p_i),
                        op0=mybir.AluOpType.add,
                        op1=mybir.AluOpType.subtract,
                        accum_out=A2[:, p_i : p_i + 1],
                    )
                so = scr_pool.tile([P, 2 * BLK], BF16, tag="scr", name=f"so{t}")
                # both passes share s_j, so one fused square-sum suffices:
                # A1[:, t] = sum over all passes of u^2
                nc.scalar.activation(
                    so[:, : npass * BLK],
                    u[:],
                    mybir.ActivationFunctionType.Square,
                    accum_out=A1[:, t : t + 1],
                )

        nc.sync.dma_start(a1_d[:], A1[:])
        nc.sync.dma_start(a2_d[:], A2[:])
        nc.sync.dma_start(sn_d[0:4, :], sT_bfs[0][:])
        nc.sync.dma_start(sn_d[4:8, :], sT_bfs[1][:])

    nc.compile()
    return nc


def shard_inputs(embeddings: np.ndarray, art_dist: np.ndarray):
    bf16 = ml_dtypes.bfloat16
    f8 = ml_dtypes.float8_e4m3
    Eb = embeddings.astype(bf16)
    Ebf = Eb.astype(np.float32)
    ident = np.eye(P, dtype=np.float32)

    def pack_kxf(M, width):  # [rows, width] -> k-tiled [128, (rows//128)*width]
        r = M.shape[0]
        return M.reshape(r // P, P, width).transpose(1, 0, 2).reshape(P, -1)

    in_maps = []
    for c in range(NCORES):
        jobs = subjobs(c)
        lhs_blocks = [c, c % 4]
        # lhs: (-2 E_blk)^T k-tiled, 2 slots
        lhs = np.concatenate(
            [
                pack_kxf((-2.0 * Ebf[b * BLK : (b + 1) * BLK]).astype(bf16).T, BLK)
                for b in lhs_blocks
            ],
            axis=1,
        )
        # eslab: row-major E for the 2 lhs blocks
        esl = np.concatenate(
            [pack_kxf(Eb[b * BLK : (b + 1) * BLK], D) for b in lhs_blocks], axis=1
        )
        # rhs: one [768, 128] k-tiled chunk per sub-job (the j-tile columns)
        rhs = np.concatenate(
            [
                pack_kxf(
                    np.ascontiguousarray(
                        Eb.T[:, bj * BLK + jt * P : bj * BLK + (jt + 1) * P]
                    ),
                    P,
                )
                for (bi, bj, jt, npass) in jobs
            ],
            axis=1,
        )
        # art: one [128 j, 512 i] tile per pass, in pass order
        tiles = []
        for (bi, bj, jt, npass) in jobs:
            i_sl = slice(bi * BLK, (bi + 1) * BLK)
            j_sl = slice(bj * BLK + jt * P, bj * BLK + (jt + 1) * P)
            if npass == 1:
                tiles.append(art_dist[j_sl, i_sl])
            else:
                tiles.append(art_dist[i_sl, j_sl].T)
                tiles.append(art_dist[j_sl, i_sl])
        art = np.concatenate([t.astype(f8) for t in tiles], axis=1)
        in_maps.append(
            {
                "lhs": np.ascontiguousarray(lhs),
                "rhs": np.ascontiguousarray(rhs),
                "eslab": np.ascontiguousarray(esl),
                "art": np.ascontiguousarray(art),
                "ident": ident,
            }
        )
    return in_maps


def combine(results):
    """Host unshard: loss from per-core A1/A2 and device-computed norms."""
    s_glob = np.zeros(B, np.float64)
    for c in range(NCORES):
        s_glob[c * BLK : (c + 1) * BLK] = (
            results[c]["snorm"][0:4].astype(np.float64).reshape(BLK)
        )
    total = 0.0
    for c in range(NCORES):
        A1 = results[c]["a1"].astype(np.float64)
        A2 = results[c]["a2"].astype(np.float64)
        pi = 0
        for t, (bi, bj, jt, npass) in enumerate(subjobs(c)):
            sj = s_glob[bj * BLK + jt * P : bj * BLK + (jt + 1) * P]
            a2sum = A2[:, pi : pi + npass].sum(axis=1)
            total += (A1[:, t] + 2 * sj * a2sum + npass * BLK * sj * sj).sum()
            pi += npass
    return np.float32(total / PAIRS)


def _get_nc():
    if "nc" not in _CACHED:
        _CACHED["nc"] = build_graph()
    return _CACHED["nc"]


def _ensure_ntff_hook():
    """The agent image's antenv package lacks axon_hooks, so trace=True in
    run_bass_kernel_spmd crashes on import. Recreate the module + register
    the ctypes NTFF hook the way trn_boot would have."""
    try:
        from antenv.axon_hooks import get_axon_ntff_profile_hook  # noqa: F401

        return
    except ImportError:
        pass
    import types

    import antenv

    mod = types.ModuleType("antenv.axon_hooks")
    holder = {"hook": None}
    mod.set_axon_ntff_profile_hook = lambda h: holder.__setitem__("hook", h)
    mod.get_axon_ntff_profile_hook = lambda: holder["hook"]
    sys.modules["antenv.axon_hooks"] = mod
    antenv.axon_hooks = mod
    try:
        from trn_agent_boot.trn_boot import _ntff_profile_via_ctypes

        for so in ("/opt/axon/libaxon_pjrt.so",):
            if os.path.exists(so):
                holder["hook"] = _ntff_profile_via_ctypes(so)
                break
    except Exception as e:  # degrade: tracing skipped, run still works
        print(f"ntff hook setup failed ({e}); tracing disabled", file=sys.stderr)


def run(embeddings: np.ndarray, art_dist: np.ndarray, **run_kwargs):
    if run_kwargs.get("trace"):
        _ensure_ntff_hook()
    nc = _get_nc()
    in_maps = shard_inputs(np.asarray(embeddings), np.asarray(art_dist))
    res = run_bass_kernel_spmd(nc, in_maps, core_ids=list(range(NCORES)), **run_kwargs)
    loss = combine(res.results)
    return np.asarray(loss, dtype=np.float32), res


def kernel(embeddings: np.ndarray, art_dist: np.ndarray) -> np.ndarray:
    loss, _ = run(embeddings, art_dist)
    return loss
